# revision 18
# baseline (speedup 1.0000x reference)
"""CliffordLinear kernel for Trainium2 (8 NeuronCores, data parallel).

The reference applies 2016 sequential Givens rotations (dim=64) to every row
of x, then adds a bias. The sequence composes into one 64x64 orthogonal
matrix R, so out = x @ R + bias. The coeffs are ~0.01 so R is near identity:
out = x + x @ A + bias with A = R - I and ||x@A|| ~ 0.08*||x||.

The device pass is HBM-bound (~358 GB/s per NeuronCore shared by loads and
stores), so the only lever is bytes. The residual split makes fp8 viable for
BOTH streams: the device computes only delta = x @ A with x, A, and delta all
in fp8-e4m3 (TRN FP8_EXP4, max +-240); the host adds x + delta/sd + bias in
fp32. Input-quantization noise passes through A (12x contraction in norm),
and delta's own quantization is 12x smaller than out, so total rel err is
~3.7e-3 versus the 2e-2 gate while DMA traffic halves versus fp16
(4 MiB in + 4 MiB out per core -> ~23.4 us roofline).

Scaling: x8 = e4m3(16*x), W8 = e4m3(8*blockdiag(A,A)), PSUM = 128*(x@A)
(max ~71, inside e4m3 normal range), drain is a pure fp32->fp8 convert-copy
(no bias on device), host divides by 128.

Device layout matches the fp16 baseline: partition p = b*64+d holds feature
d of row-block b (two 32768-row blocks stacked), W is block-diagonal so one
[128,128] stationary matmul processes both blocks with all partitions
active. Tiles [128, 4096] fp8 are tile-major in DRAM so every DMA is one
contiguous 512 KiB block. Per tile, 8 matmuls of 512 cols accumulate into
PSUM banks; PSUM->SBUF drains (fp32->fp8 convert) alternate between the
Vector and Scalar engines. Loads ride the SP HWDGE ring and stores the
GPSIMD SWDGE ring, keeping store issue off both drain engines and letting
next-rep loads bypass store semaphore waits (measured ~2-3 us faster than
stores on the ACT or SP rings).
"""

import numpy as np

DIM = 64
NROWS = 524288
NCORES = 8
SHARD = NROWS // NCORES  # 65536 rows per core
HALF = SHARD // 2        # 32768 columns per stacked block
TILE_COLS = 4096         # columns per DMA tile (128*4096*1 = 512 KiB fp8)
MM_COLS = 512            # moving-operand columns per matmul (one PSUM bank)

SX = 16.0                # x pre-scale into fp8
SA = 8.0                 # A pre-scale into fp8
SD = SX * SA             # delta comes back scaled by SD

_BASS_CACHE = {}


def _f8_dtype():
    import ml_dtypes

    return ml_dtypes.float8_e4m3


def _compose_rotation(coeffs64):
    """R such that applying the reference rotation sequence == x @ R."""
    ii, jj = np.triu_indices(DIM, k=1)
    c = np.cos(coeffs64)
    s = np.sin(coeffs64)
    R = np.eye(DIM, dtype=np.float64)
    for k in range(len(ii)):
        i, j = int(ii[k]), int(jj[k])
        ri = R[:, i].copy()
        rj = R[:, j].copy()
        R[:, i] = c[k] * ri - s[k] * rj
        R[:, j] = s[k] * ri + c[k] * rj
    return R


def _pack_shard(xs, tile_cols):
    """(SHARD, DIM) fp32 -> [T, 128, tile_cols] fp8 tile-major layout."""
    dt = _f8_dtype()
    t = HALF // tile_cols
    xq = np.clip(xs * np.float32(SX), -240.0, 240.0).astype(dt)
    x2 = xq.reshape(2, HALF, DIM).transpose(0, 2, 1).reshape(128, HALF)
    return np.ascontiguousarray(
        x2.reshape(128, t, tile_cols).transpose(1, 0, 2)
    )


def _unpack_shard(o3, tile_cols, xs, bias):
    """[T, 128, tile_cols] fp8 delta -> (SHARD, DIM) fp32 out = x+delta+b."""
    o2 = np.asarray(o3).transpose(1, 0, 2).reshape(128, HALF)
    delta = o2.reshape(2, DIM, HALF).transpose(0, 2, 1).reshape(
        SHARD, DIM).astype(np.float32)
    delta *= np.float32(1.0 / SD)
    delta += xs
    delta += bias
    return delta


def _build_bass(half=HALF, tile_cols=TILE_COLS, n_cores=NCORES, reps=1,
                io_bufs=6, ps_bufs=8, mm_cols=MM_COLS, tiny_out=False,
                tiny_in=False, x_bufs=1, drain_cols=512, dve_of=(8, 16),
                store_cols=None, store_eng="gpsimd", load_eng="sync",
                stages="full", store_tile_cols=None, dve_start=True,
                drain_pat="alt"):
    import concourse.bass as bass
    import concourse.bacc as bacc
    import concourse.mybir as mybir
    import concourse.tile as tile

    f8 = mybir.dt.float8e4
    nc = bacc.Bacc(
        "TRN2", target_bir_lowering=False, debug=False, num_devices=n_cores
    )
    n_tiles = half // tile_cols
    mm_per_drain = drain_cols // mm_cols
    # store tiling is decoupled from load tiling: bigger store tiles
    # amortize the SWDGE per-op fixed cost while loads stay fine-grained
    store_tile_cols = store_tile_cols or (store_cols or tile_cols)
    n_stiles = half // store_tile_cols

    # tiny_in: timing builds read x2 from an Internal DRAM scratch tensor
    # (uninitialized — DMA/compute time is value-independent) so the 4 MiB
    # per-core input never crosses the axon tunnel per timed call.
    x_d = nc.dram_tensor("x2", [n_tiles, 128, tile_cols], f8,
                         kind="Internal" if tiny_in else "ExternalInput")
    w_d = nc.dram_tensor("w", [128, 128], f8, kind="ExternalInput")
    # tiny_out: timing builds keep every DMA/compute identical but land o2 in
    # an Internal DRAM scratch tensor, exposing only a tiny real output —
    # returning the full output per call through the axon tunnel costs an
    # unstable 10-80 ms that swamps the per-rep timing signal.
    o_d = nc.dram_tensor("o2", [n_stiles, 128, store_tile_cols], f8,
                         kind="Internal" if tiny_out else "ExternalOutput")
    s_d = (nc.dram_tensor("osmall", [128, 1], f8, kind="ExternalOutput")
           if tiny_out else None)

    with tile.TileContext(nc) as tc:
        with (
            tc.tile_pool(name="const", bufs=1) as cpool,
            tc.tile_pool(name="io", bufs=io_bufs) as iopool,
            tc.tile_pool(name="xp", bufs=x_bufs) as xpool,
            tc.tile_pool(name="ps", bufs=ps_bufs,
                         space=bass.MemorySpace.PSUM) as pspool,
        ):
            w = cpool.tile([128, 128], f8)
            nc.sync.dma_start(w[:], w_d[:])
            for _rep in range(reps):
                xins = []
                le = {"sync": nc.sync, "scalar": nc.scalar,
                      "gpsimd": nc.gpsimd}.get(load_eng)
                for t in range(n_tiles):
                    xin = xpool.tile([128, tile_cols], f8, tag=f"xin{t}")
                    eng = le if le is not None else (
                        nc.sync if t % 2 == 0 else nc.scalar)
                    eng.dma_start(xin[:], x_d[t])
                    xins.append(xin)
                # weighted round-robin: DVE takes dve_of[0] of every
                # dve_of[1] drains (measured per-chunk cost is ~equal on
                # DVE and ACT here, so the default is an even alternation)
                dve_err = dve_of[1] - 1 if dve_start else 0
                st = {"scalar": nc.scalar, "sync": nc.sync,
                      "gpsimd": nc.gpsimd}[store_eng]
                out = None
                for g in range(half // drain_cols):
                    col = g * drain_cols
                    t = col // tile_cols
                    if col % store_tile_cols == 0:
                        out = iopool.tile([128, store_tile_cols], f8,
                                          tag="out")
                    base_x = col % tile_cols
                    base_o = col % store_tile_cols
                    ps = pspool.tile([128, drain_cols], mybir.dt.float32)
                    for u in range(mm_per_drain):
                        nc.tensor.matmul(
                            ps[:, u * mm_cols:(u + 1) * mm_cols],
                            w[:],
                            xins[t][:, base_x + u * mm_cols:
                                    base_x + (u + 1) * mm_cols],
                            start=True,
                            stop=True,
                        )
                    oc = out[:, base_o:base_o + drain_cols]
                    if drain_pat == "pair":
                        use_dve = (g // 2) % 2 == 0
                    else:
                        dve_err += dve_of[0]
                        use_dve = dve_err >= dve_of[1]
                        if use_dve:
                            dve_err -= dve_of[1]
                    if stages in ("lmd", "full"):
                        if use_dve:
                            nc.vector.tensor_copy(oc, ps[:])
                        else:
                            nc.scalar.copy(oc, ps[:])
                    if stages == "full" and \
                            (col + drain_cols) % store_tile_cols == 0:
                        st.dma_start(o_d[col // store_tile_cols], out[:])
            if s_d is not None:
                # osmall must depend on the o2 store stream, or the call
                # "completes" while stores still drain and the timing hides
                # the tail: read BACK from o2 on the store ring (per-engine
                # program order puts this after the last store), then export
                sm = cpool.tile([128, 1], f8, tag="osmall")
                nc.scalar.dma_start(sm[:], o_d[n_stiles - 1][:, 0:1])
                nc.scalar.dma_start(s_d[:], sm[:])
    nc.compile()
    return nc


def _make_w8(A):
    dt = _f8_dtype()
    W = np.zeros((128, 128), dtype=np.float32)
    W[:DIM, :DIM] = A * SA
    W[DIM:, DIM:] = A * SA
    return np.clip(W, -240.0, 240.0).astype(dt)


def kernel(x, bivector_coeffs, bias):
    from concourse.bass_utils import run_bass_kernel_spmd

    x = np.ascontiguousarray(np.asarray(x, dtype=np.float32))
    coeffs = np.asarray(bivector_coeffs, dtype=np.float64)
    bias = np.asarray(bias, dtype=np.float32)

    R = _compose_rotation(coeffs)
    A = R - np.eye(DIM)
    W8 = _make_w8(A)

    key = (HALF, TILE_COLS, NCORES, "f8e4")
    if key not in _BASS_CACHE:
        _BASS_CACHE[key] = _build_bass(
            half=HALF, tile_cols=TILE_COLS, n_cores=NCORES, reps=1,
        )
    nc = _BASS_CACHE[key]

    in_maps = []
    for r in range(NCORES):
        xs = x[r * SHARD:(r + 1) * SHARD]
        in_maps.append({"x2": _pack_shard(xs, TILE_COLS), "w": W8})

    res = run_bass_kernel_spmd(
        nc, in_maps, core_ids=list(range(NCORES)), trace=False
    )

    out = np.empty((NROWS, DIM), dtype=np.float32)
    for r in range(NCORES):
        out[r * SHARD:(r + 1) * SHARD] = _unpack_shard(
            res.results[r]["o2"], TILE_COLS, x[r * SHARD:(r + 1) * SHARD],
            bias,
        )
    return out


# revision 19
# speedup vs baseline: 1.0105x; 1.0105x over previous
"""CliffordLinear kernel for Trainium2 (8 NeuronCores, data parallel).

The reference applies 2016 sequential Givens rotations (dim=64) to every row
of x, then adds a bias. The sequence composes into one 64x64 orthogonal
matrix R, so out = x @ R + bias. The coeffs are ~0.01 so R is near identity:
out = x + x @ A + bias with A = R - I and ||x@A|| ~ 0.08*||x||.

The device pass is HBM-bound (~358 GB/s per NeuronCore shared by loads and
stores), so the only lever is bytes. The residual split makes fp8 viable for
BOTH streams: the device computes only delta = x @ A with x, A, and delta all
in fp8-e4m3 (TRN FP8_EXP4, max +-240); the host adds x + delta/sd + bias in
fp32. Input-quantization noise passes through A (12x contraction in norm),
and delta's own quantization is 12x smaller than out, so total rel err is
~3.7e-3 versus the 2e-2 gate while DMA traffic halves versus fp16
(4 MiB in + 4 MiB out per core -> ~23.4 us roofline).

Scaling: x8 = e4m3(16*x), W8 = e4m3(8*blockdiag(A,A)), PSUM = 128*(x@A)
(max ~71, inside e4m3 normal range), drain is a pure fp32->fp8 convert-copy
(no bias on device), host divides by 128.

Device layout matches the fp16 baseline: partition p = b*64+d holds feature
d of row-block b (two 32768-row blocks stacked), W is block-diagonal so one
[128,128] stationary matmul processes both blocks with all partitions
active. Tiles [128, 4096] fp8 are tile-major in DRAM so every DMA is one
contiguous 512 KiB block. Per tile, 8 matmuls of 512 cols accumulate into
PSUM banks; PSUM->SBUF drains (fp32->fp8 convert) alternate between the
Vector and Scalar engines. Loads ride the SP HWDGE ring and stores the
GPSIMD SWDGE ring, keeping store issue off both drain engines and letting
next-rep loads bypass store semaphore waits (measured ~2-3 us faster than
stores on the ACT or SP rings).
"""

import numpy as np

DIM = 64
NROWS = 524288
NCORES = 8
SHARD = NROWS // NCORES  # 65536 rows per core
HALF = SHARD // 2        # 32768 columns per stacked block
TILE_COLS = 4096         # columns per DMA tile (128*4096*1 = 512 KiB fp8)
MM_COLS = 512            # moving-operand columns per matmul (one PSUM bank)

SX = 16.0                # x pre-scale into fp8
SA = 8.0                 # A pre-scale into fp8
SD = SX * SA             # delta comes back scaled by SD

_BASS_CACHE = {}


def _f8_dtype():
    import ml_dtypes

    return ml_dtypes.float8_e4m3


def _compose_rotation(coeffs64):
    """R such that applying the reference rotation sequence == x @ R."""
    ii, jj = np.triu_indices(DIM, k=1)
    c = np.cos(coeffs64)
    s = np.sin(coeffs64)
    R = np.eye(DIM, dtype=np.float64)
    for k in range(len(ii)):
        i, j = int(ii[k]), int(jj[k])
        ri = R[:, i].copy()
        rj = R[:, j].copy()
        R[:, i] = c[k] * ri - s[k] * rj
        R[:, j] = s[k] * ri + c[k] * rj
    return R


def _pack_shard(xs, tile_cols):
    """(SHARD, DIM) fp32 -> [T, 128, tile_cols] fp8 tile-major layout."""
    dt = _f8_dtype()
    t = HALF // tile_cols
    xq = np.clip(xs * np.float32(SX), -240.0, 240.0).astype(dt)
    x2 = xq.reshape(2, HALF, DIM).transpose(0, 2, 1).reshape(128, HALF)
    return np.ascontiguousarray(
        x2.reshape(128, t, tile_cols).transpose(1, 0, 2)
    )


def _unpack_shard(o3, tile_cols, xs, bias):
    """[T, 128, tile_cols] fp8 delta -> (SHARD, DIM) fp32 out = x+delta+b."""
    o2 = np.asarray(o3).transpose(1, 0, 2).reshape(128, HALF)
    delta = o2.reshape(2, DIM, HALF).transpose(0, 2, 1).reshape(
        SHARD, DIM).astype(np.float32)
    delta *= np.float32(1.0 / SD)
    delta += xs
    delta += bias
    return delta


def _build_bass(half=HALF, tile_cols=TILE_COLS, n_cores=NCORES, reps=1,
                io_bufs=6, ps_bufs=8, mm_cols=MM_COLS, tiny_out=False,
                tiny_in=False, x_bufs=1, drain_cols=512, dve_of=(8, 16),
                store_cols=None, store_eng="gpsimd", load_eng="sync",
                stages="full", store_tile_cols=None, dve_start=True,
                drain_pat="alt"):
    import concourse.bass as bass
    import concourse.bacc as bacc
    import concourse.mybir as mybir
    import concourse.tile as tile

    f8 = mybir.dt.float8e4
    nc = bacc.Bacc(
        "TRN2", target_bir_lowering=False, debug=False, num_devices=n_cores
    )
    n_tiles = half // tile_cols
    mm_per_drain = drain_cols // mm_cols
    # store tiling is decoupled from load tiling: bigger store tiles
    # amortize the SWDGE per-op fixed cost while loads stay fine-grained
    store_tile_cols = store_tile_cols or (store_cols or tile_cols)
    n_stiles = half // store_tile_cols

    # tiny_in: timing builds read x2 from an Internal DRAM scratch tensor
    # (uninitialized — DMA/compute time is value-independent) so the 4 MiB
    # per-core input never crosses the axon tunnel per timed call.
    x_d = nc.dram_tensor("x2", [n_tiles, 128, tile_cols], f8,
                         kind="Internal" if tiny_in else "ExternalInput")
    w_d = nc.dram_tensor("w", [128, 128], f8, kind="ExternalInput")
    # tiny_out: timing builds keep every DMA/compute identical but land o2 in
    # an Internal DRAM scratch tensor, exposing only a tiny real output —
    # returning the full output per call through the axon tunnel costs an
    # unstable 10-80 ms that swamps the per-rep timing signal.
    o_d = nc.dram_tensor("o2", [n_stiles, 128, store_tile_cols], f8,
                         kind="Internal" if tiny_out else "ExternalOutput")
    s_d = (nc.dram_tensor("osmall", [128, 1], f8, kind="ExternalOutput")
           if tiny_out else None)

    with tile.TileContext(nc) as tc:
        with (
            tc.tile_pool(name="const", bufs=1) as cpool,
            tc.tile_pool(name="io", bufs=io_bufs) as iopool,
            tc.tile_pool(name="xp", bufs=x_bufs) as xpool,
            tc.tile_pool(name="ps", bufs=ps_bufs,
                         space=bass.MemorySpace.PSUM) as pspool,
        ):
            w = cpool.tile([128, 128], f8)
            nc.sync.dma_start(w[:], w_d[:])
            for _rep in range(reps):
                xins = []
                le = {"sync": nc.sync, "scalar": nc.scalar,
                      "gpsimd": nc.gpsimd}.get(load_eng)
                for t in range(n_tiles):
                    xin = xpool.tile([128, tile_cols], f8, tag=f"xin{t}")
                    eng = le if le is not None else (
                        nc.sync if t % 2 == 0 else nc.scalar)
                    eng.dma_start(xin[:], x_d[t])
                    xins.append(xin)
                # weighted round-robin: DVE takes dve_of[0] of every
                # dve_of[1] drains (measured per-chunk cost is ~equal on
                # DVE and ACT here, so the default is an even alternation)
                dve_err = dve_of[1] - 1 if dve_start else 0
                st = {"scalar": nc.scalar, "sync": nc.sync,
                      "gpsimd": nc.gpsimd}.get(store_eng, nc.gpsimd)
                out = None
                for g in range(half // drain_cols):
                    col = g * drain_cols
                    t = col // tile_cols
                    if col % store_tile_cols == 0:
                        out = iopool.tile([128, store_tile_cols], f8,
                                          tag="out")
                    base_x = col % tile_cols
                    base_o = col % store_tile_cols
                    ps = pspool.tile([128, drain_cols], mybir.dt.float32)
                    for u in range(mm_per_drain):
                        nc.tensor.matmul(
                            ps[:, u * mm_cols:(u + 1) * mm_cols],
                            w[:],
                            xins[t][:, base_x + u * mm_cols:
                                    base_x + (u + 1) * mm_cols],
                            start=True,
                            stop=True,
                        )
                    oc = out[:, base_o:base_o + drain_cols]
                    if drain_pat == "pair":
                        use_dve = (g // 2) % 2 == 0
                    else:
                        dve_err += dve_of[0]
                        use_dve = dve_err >= dve_of[1]
                        if use_dve:
                            dve_err -= dve_of[1]
                    if stages in ("lmd", "full"):
                        if use_dve:
                            nc.vector.tensor_copy(oc, ps[:])
                        else:
                            nc.scalar.copy(oc, ps[:])
                    if stages == "full" and \
                            (col + drain_cols) % store_tile_cols == 0:
                        if store_eng == "split2":
                            h = store_tile_cols // 2
                            sti = col // store_tile_cols
                            nc.gpsimd.dma_start(o_d[sti][:, :h], out[:, :h])
                            nc.scalar.dma_start(o_d[sti][:, h:], out[:, h:])
                        else:
                            st.dma_start(o_d[col // store_tile_cols], out[:])
            if s_d is not None:
                # osmall must depend on the o2 store stream, or the call
                # "completes" while stores still drain and the timing hides
                # the tail: read BACK from o2 on the store ring (per-engine
                # program order puts this after the last store), then export
                sm = cpool.tile([128, 1], f8, tag="osmall")
                nc.scalar.dma_start(sm[:], o_d[n_stiles - 1][:, 0:1])
                nc.scalar.dma_start(s_d[:], sm[:])
    nc.compile()
    return nc


def _make_w8(A):
    dt = _f8_dtype()
    W = np.zeros((128, 128), dtype=np.float32)
    W[:DIM, :DIM] = A * SA
    W[DIM:, DIM:] = A * SA
    return np.clip(W, -240.0, 240.0).astype(dt)


def kernel(x, bivector_coeffs, bias):
    from concourse.bass_utils import run_bass_kernel_spmd

    x = np.ascontiguousarray(np.asarray(x, dtype=np.float32))
    coeffs = np.asarray(bivector_coeffs, dtype=np.float64)
    bias = np.asarray(bias, dtype=np.float32)

    R = _compose_rotation(coeffs)
    A = R - np.eye(DIM)
    W8 = _make_w8(A)

    key = (HALF, TILE_COLS, NCORES, "f8e4")
    if key not in _BASS_CACHE:
        _BASS_CACHE[key] = _build_bass(
            half=HALF, tile_cols=TILE_COLS, n_cores=NCORES, reps=1,
        )
    nc = _BASS_CACHE[key]

    in_maps = []
    for r in range(NCORES):
        xs = x[r * SHARD:(r + 1) * SHARD]
        in_maps.append({"x2": _pack_shard(xs, TILE_COLS), "w": W8})

    res = run_bass_kernel_spmd(
        nc, in_maps, core_ids=list(range(NCORES)), trace=False
    )

    out = np.empty((NROWS, DIM), dtype=np.float32)
    for r in range(NCORES):
        out[r * SHARD:(r + 1) * SHARD] = _unpack_shard(
            res.results[r]["o2"], TILE_COLS, x[r * SHARD:(r + 1) * SHARD],
            bias,
        )
    return out
